# revision 3
# baseline (speedup 1.0000x reference)
"""GRU kernel for Trainium2, 8 NeuronCores, data-parallel over batch.

Reference computation (per timestep, batch-major):
    z = sigmoid(x_t @ W_z + s @ R_z + B_z)
    r = sigmoid(x_t @ W_r + s @ R_r + B_r)
    h = tanh   (x_t @ W_h + (r*s) @ R_h + B_h)
    s = (1-z)*s + z*h
Returns final s: [B, H].

Shapes: B=128, T=1024, D=512, H=1024.  Sharding: batch 16 per core.

Kernel design (per core):
  Phase A: XP2[t, b, 3H] = x @ [W_r|W_z|W_h] + B precomputed at full PE
           efficiency (M=128 tiles), stored to internal DRAM time-major.
  Phase B: sequential scan. Gate pre-activations accumulate in PSUM
           batch-major [16, 1024] (xp seeded via K=128 selection-matrix
           injects, then 8 K-chunk matmuls with the H-major state as the
           stationary operand). Each gate is processed in 512-col halves
           so sigmoid/tanh (ScalarE) overlap the second half's matmuls.
           All elementwise state math runs H-major on [128, 64..128]
           tiles (VectorE free-dim cost ~64-128, not 1024); the state is
           born H-major so it feeds the next step's matmuls directly.
"""

import numpy as np

import concourse.bass as bass
from concourse import bacc
import concourse.mybir as mybir
from concourse.tile import TileContext
from concourse.bass_utils import run_bass_kernel_spmd
from concourse.masks import make_identity

B, T, D, H = 128, 1024, 512, 1024
NCORES = 8
BC = B // NCORES          # 16 batch rows per core
H3 = 3 * H                # gates concatenated [r|z|h]
KD = D // 128             # 4 k-chunks over input features
KH = H // 128             # 8 k-chunks over hidden dim
FP = mybir.dt.float32
FPR = mybir.dt.float32r
AF = mybir.ActivationFunctionType
OP = mybir.AluOpType

def _r(ap):
    return ap.bitcast(FPR)


def _f(ap):
    return ap.bitcast(FP)


def build_gru(t_steps=T):
    nc = bacc.Bacc()
    xT = nc.declare_dram_parameter("xT", [D, BC * t_steps], FPR, False)
    Wc = nc.declare_dram_parameter("Wcat", [D, H3], FPR, False)
    Bc = nc.declare_dram_parameter("Bcat", [128, H3], FP, False)
    Rc = nc.declare_dram_parameter("Rcat", [H, H3], FPR, False)
    out = nc.declare_dram_parameter("out", [BC, H], FP, True)
    XP2 = nc.dram_tensor("XP2", [t_steps, BC, H3], FPR)   # time-major

    MT = (BC * t_steps) // 128   # 128-row tiles over (b, t)
    NT = H3 // 512               # 6 n-tiles of 512

    with TileContext(nc) as tc:
        with tc.tile_pool(name="const_pool", bufs=1) as cp:
            # identity [128,128] for full-tile transposes
            i128_t = cp.tile([128, 128], FP)
            make_identity(nc, i128_t[:])
            i128 = cp.tile([128, 128], FPR)
            nc.scalar.copy(out=i128[:], in_=i128_t[:])
            # injects contract the 4-timestep-packed xp tile (64 rows) with
            # a 16-column slice of I_64: column block ts selects rows
            # 16*ts..16*ts+16
            identsel = cp.tile([64, 64], FPR)
            nc.scalar.copy(out=identsel[:], in_=i128_t[0:64, 0:64])

            # ---------------- phase A: XP2 = x @ Wcat + B ----------------
            with (
                tc.tile_pool(name="phase_a_w", bufs=1) as wp,
                tc.tile_pool(name="a_x", bufs=4) as axp,
                tc.tile_pool(name="a_ps", bufs=4, space="PSUM") as aps,
                tc.tile_pool(name="a_out", bufs=4) as aop,
            ):
                bias_bc = wp.tile([128, H3], FP)
                nc.sync.dma_start(out=bias_bc[:], in_=Bc[:, :])
                w_sb = wp.tile([128, KD * H3], FPR)
                nc.sync.dma_start(
                    out=w_sb[:],
                    in_=Wc[:].rearrange("(kd p) n -> p kd n", kd=KD),
                )
                xT_v = xT[:].rearrange("(kd p) m -> p kd m", kd=KD)
                for mt in range(MT):
                    x_sb = axp.tile([128, KD * 128], FPR)
                    nc.sync.dma_start(
                        out=x_sb[:],
                        in_=xT_v[:, :, mt * 128:(mt + 1) * 128],
                    )
                    for ntile in range(NT):
                        ps = aps.tile([128, 512], FP, tag="a_ps")
                        for kd in range(KD):
                            nc.tensor.matmul(
                                ps[:],
                                x_sb[:, kd * 128:(kd + 1) * 128],
                                w_sb[:, kd * H3 + ntile * 512:
                                     kd * H3 + (ntile + 1) * 512],
                                start=(kd == 0),
                                stop=(kd == KD - 1),
                            )
                        o_sb = aop.tile([128, 512], FPR)
                        nc.vector.tensor_tensor(
                            o_sb[:], ps[:],
                            bias_bc[:, ntile * 512:(ntile + 1) * 512], OP.add,
                        )
                        nsl = slice(ntile * 512, (ntile + 1) * 512)
                        if t_steps >= 128:
                            tj = t_steps // 128
                            b, j = mt // tj, mt % tj
                            dst = XP2[128 * j:128 * (j + 1), b, nsl]
                        else:
                            nb = 128 // t_steps
                            b0 = mt * nb
                            dst = XP2[:, b0:b0 + nb, nsl].rearrange(
                                "t b n -> b t n")
                        nc.sync.dma_start(out=dst, in_=o_sb[:])

            # ---------------- phase B: the scan ----------------
            with (
                tc.tile_pool(name="scan_r", bufs=1) as rp,
                tc.tile_pool(name="xp_pool", bufs=2) as xpp,
                tc.tile_pool(name="sb_pool", bufs=2) as sbp,
                tc.tile_pool(name="ps_pool", bufs=1, space="PSUM") as psp,
            ):
                scan_body(nc, tc, rp, xpp, sbp, psp, i128, identsel, Rc, XP2,
                          out, t_steps)
    nc.finalize()
    return nc


def scan_body(nc, tc, rp, xpp, sbp, psp, i128, identsel, Rc, XP2, out, t_steps):
    R_sb = rp.tile([128, KH * H3], FPR)   # 96KB/partition, resident
    nc.sync.dma_start(
        out=R_sb[:],
        in_=Rc[:].rearrange("(kh p) n -> p kh n", kh=KH),
    )

    # psum: gate pre-activations [16, 1024] batch-major (2 banks each),
    # one shared bank for the three H-major transpose targets, one spare
    pr = psp.tile([16, H], FP, tag="pr")
    pz = psp.tile([16, H], FP, tag="pz")
    ph = psp.tile([16, H], FP, tag="ph")
    tps = psp.tile([128, 512], FPR, tag="tps")
    tps_r, tps_z, tps_h = tps[:, 0:128], tps[:, 128:256], tps[:, 256:384]
    gate_ps = {0: pr, 1: pz, 2: ph}
    for t_ in (pr, pz, ph):
        nc.vector.memset(t_[:], 0.0)

    zini = sbp.tile([128, 128], FP, tag="zini", bufs=1)
    nc.gpsimd.memset(zini[:], 0.0)
    sT0 = sbp.tile([128, 128], FPR, tag="sT")
    nc.vector.tensor_copy(sT0[:], zini[:])

    NB = max(t_steps // 4, 1)
    xp_tiles = {}

    def xp_dma(blk):
        xp4 = xpp.tile([64, H3], FPR, tag="xp4")
        nc.sync.dma_start(
            out=xp4[:],
            in_=XP2[blk * 4:blk * 4 + 4, :, :].rearrange("t b n -> (t b) n"))
        xp_tiles[blk] = xp4

    def inject(t, gate):
        """Seed psum[gate] for step t with XP(+bias): 2 n-halves."""
        xp4 = xp_tiles[t // 4]
        ts = t % 4
        ps = gate_ps[gate]
        for nt in range(2):
            nc.tensor.matmul(
                ps[:, nt * 512:(nt + 1) * 512],
                identsel[:, ts * 16:ts * 16 + 16],
                xp4[:, gate * H + nt * 512: gate * H + (nt + 1) * 512],
                start=True, stop=False,
            )

    def gate_mms(gate, stat, nt):
        """One 512-col half of gate += stat.T @ R[:, gate]; 8 K-chunks."""
        ps = gate_ps[gate]
        for k in range(KH):
            nc.tensor.matmul(
                ps[:, nt * 512:(nt + 1) * 512],
                stat[:, 16 * k:16 * k + 16],
                R_sb[:, k * H3 + gate * H + nt * 512:
                     k * H3 + gate * H + (nt + 1) * 512],
                start=False, stop=(k == KH - 1),
            )

    def transp(dst, src_sb, half):
        """4 chunk transposes of one 512-col half into H-major dst."""
        for k in range(4 * half, 4 * half + 4):
            nc.tensor.transpose(
                dst[:, 16 * k:16 * k + 16],
                src_sb[0:16, 128 * k:128 * (k + 1)],
                i128[0:16, 0:16],
            )

    # ---- prologue ----
    for blk in range(min(2, NB)):
        xp_dma(blk)
    sT = sT0
    for gate in range(3):
        inject(0, gate)

    for t in range(t_steps):
        last = t + 1 >= t_steps
        # ---- r gate ----
        gate_mms(0, sT, 0)
        gate_mms(0, sT, 1)
        r_sb = sbp.tile([16, H], FPR, tag="r_sb")
        nc.scalar.activation(r_sb[:, 0:512], pr[:, 0:512], AF.Sigmoid)
        transp(tps_r, r_sb, 0)
        nc.scalar.activation(r_sb[:, 512:], pr[:, 512:], AF.Sigmoid)
        transp(tps_r, r_sb, 1)
        rs = sbp.tile([128, 128], FPR, tag="rs")
        for hf in range(2):
            sl = slice(64 * hf, 64 * (hf + 1))
            nc.vector.tensor_tensor(rs[:, sl], _f(tps_r)[:, sl], _f(sT)[:, sl],
                                    OP.mult)

        # ---- z gate ----
        gate_mms(1, sT, 0)
        gate_mms(1, sT, 1)
        z_sb = sbp.tile([16, H], FPR, tag="z_sb")
        nc.scalar.activation(z_sb[:, 0:512], pz[:, 0:512], AF.Sigmoid)
        transp(tps_z, z_sb, 0)
        nc.scalar.activation(z_sb[:, 512:], pz[:, 512:], AF.Sigmoid)
        transp(tps_z, z_sb, 1)
        zT = sbp.tile([128, 128], FP, tag="zT")
        nc.vector.tensor_copy(zT[:], _f(tps_z))
        zs = sbp.tile([128, 128], FP, tag="zs")
        nc.vector.tensor_tensor(zs[:], zT[:], _f(sT)[:], OP.mult)
        u = sbp.tile([128, 128], FP, tag="u")
        nc.vector.tensor_tensor(u[:], _f(sT)[:], zs[:], OP.subtract)

        # ---- h gate (needs rs) ----
        gate_mms(2, rs, 0)
        gate_mms(2, rs, 1)
        if not last:
            inject(t + 1, 0)
            inject(t + 1, 1)
        h_sb = sbp.tile([16, H], FPR, tag="h_sb")
        sT_new = sbp.tile([128, 128], FPR, tag="sT")
        zh = sbp.tile([128, 128], FP, tag="zh")
        for hf in range(2):
            sl64 = slice(64 * hf, 64 * (hf + 1))
            nc.scalar.activation(h_sb[:, 512 * hf:512 * (hf + 1)],
                                 ph[:, 512 * hf:512 * (hf + 1)], AF.Tanh)
            transp(tps_h, h_sb, hf)
            nc.vector.tensor_tensor(zh[:, sl64], _f(tps_h)[:, sl64],
                                    zT[:, sl64], OP.mult)
            nc.vector.tensor_tensor(sT_new[:, sl64], u[:, sl64], zh[:, sl64],
                                    OP.add)
        if not last:
            inject(t + 1, 2)
            if t % 4 == 3 and t // 4 + 2 < NB:
                xp_dma(t // 4 + 2)
        sT = sT_new

    # ---- epilogue: H-major state -> batch-major output ----
    ops0 = psp.tile([16, 512], FPR, tag="spare")
    ops1 = psp.tile([16, 512], FPR, tag="spare")
    out_sb = sbp.tile([16, H], FP, tag="out_sb")
    for k in range(KH):
        dst = (ops0 if k < 4 else ops1)
        nc.tensor.transpose(
            dst[0:16, 128 * (k % 4):128 * (k % 4 + 1)],
            sT[:, 16 * k:16 * k + 16], i128[:],
        )
        if k == 3:
            nc.vector.tensor_copy(out_sb[:, 0:512], _f(ops0)[0:16, :])
    nc.vector.tensor_copy(out_sb[:, 512:1024], _f(ops1)[0:16, :])
    nc.sync.dma_start(out=out[:, :], in_=out_sb[:])


_CACHE = {}


def _get_nc(t_steps=T):
    key = t_steps
    if key not in _CACHE:
        _CACHE[key] = build_gru(t_steps)
    return _CACHE[key]


def prep_in_maps(inputs, t_steps=T):
    x = np.asarray(inputs["x"], dtype=np.float32)
    Wcat = np.ascontiguousarray(
        np.concatenate([np.asarray(inputs["W_r"]), np.asarray(inputs["W_z"]),
                        np.asarray(inputs["W_h"])], axis=1),
        dtype=np.float32,
    )
    Rcat = np.ascontiguousarray(
        np.concatenate([np.asarray(inputs["R_r"]), np.asarray(inputs["R_z"]),
                        np.asarray(inputs["R_h"])], axis=1),
        dtype=np.float32,
    )
    Bcat = np.ascontiguousarray(
        np.broadcast_to(
            np.concatenate([np.asarray(inputs["B_r"]), np.asarray(inputs["B_z"]),
                            np.asarray(inputs["B_h"])])[None, :], (128, H3)),
        dtype=np.float32,
    )
    in_maps = []
    for c in range(NCORES):
        xc = x[c * BC:(c + 1) * BC, :t_steps, :]          # [BC, t, D]
        xTc = np.ascontiguousarray(
            xc.transpose(2, 0, 1).reshape(D, BC * t_steps)
        )
        in_maps.append({"xT": xTc, "Wcat": Wcat, "Bcat": Bcat, "Rcat": Rcat})
    return in_maps


def assemble_output(results):
    outs = [results[c]["out"] for c in range(NCORES)]
    return np.concatenate(outs, axis=0)


def kernel_run(x, W_z, W_r, W_h, R_z, R_r, R_h, B_z, B_r, B_h, t_steps=T, **run_kw):
    in_maps = prep_in_maps(dict(x=x, W_z=W_z, W_r=W_r, W_h=W_h, R_z=R_z, R_r=R_r,
                                R_h=R_h, B_z=B_z, B_r=B_r, B_h=B_h), t_steps)
    res = run_bass_kernel_spmd(_get_nc(t_steps), in_maps, list(range(NCORES)), **run_kw)
    full = assemble_output(res.results)
    return full, res


def kernel(**inputs):
    full, _ = kernel_run(**inputs)
    return full


# revision 4
# speedup vs baseline: 1.0054x; 1.0054x over previous
"""GRU kernel for Trainium2, 8 NeuronCores, data-parallel over batch.

Reference computation (per timestep, batch-major):
    z = sigmoid(x_t @ W_z + s @ R_z + B_z)
    r = sigmoid(x_t @ W_r + s @ R_r + B_r)
    h = tanh   (x_t @ W_h + (r*s) @ R_h + B_h)
    s = (1-z)*s + z*h
Returns final s: [B, H].

Shapes: B=128, T=1024, D=512, H=1024.  Sharding: batch 16 per core.

Kernel design (per core):
  Phase A is fused into the scan: x arrives time-major, so each [128-row,
           512-col] x@W+B tile covers 8 timesteps x 16 batch and is evacuated
           from PSUM straight into the scan's SBUF xp tile (no DRAM round
           trip). Units are emitted one per step, one window ahead of the
           scan, and double as PE filler through the state-update tail.
  Phase B: sequential scan. Gate pre-activations accumulate in PSUM
           batch-major [16, 1024] (xp seeded via K=128 selection-matrix
           injects, then 8 K-chunk matmuls with the H-major state as the
           stationary operand). Each gate is processed in 512-col halves
           so sigmoid/tanh (ScalarE) overlap the second half's matmuls.
           All elementwise state math runs H-major on [128, 64..128]
           tiles (VectorE free-dim cost ~64-128, not 1024); the state is
           born H-major so it feeds the next step's matmuls directly.
"""

import numpy as np

import concourse.bass as bass
from concourse import bacc
import concourse.mybir as mybir
from concourse.tile import TileContext
from concourse.bass_utils import run_bass_kernel_spmd
from concourse.masks import make_identity

B, T, D, H = 128, 1024, 512, 1024
NCORES = 8
BC = B // NCORES          # 16 batch rows per core
H3 = 3 * H                # gates concatenated [r|z|h]
KD = D // 128             # 4 k-chunks over input features
KH = H // 128             # 8 k-chunks over hidden dim
FP = mybir.dt.float32
FPR = mybir.dt.float32r
AF = mybir.ActivationFunctionType
OP = mybir.AluOpType

def _r(ap):
    return ap.bitcast(FPR)


def _f(ap):
    return ap.bitcast(FP)


def build_gru(t_steps=T):
    nc = bacc.Bacc()
    xT = nc.declare_dram_parameter("xT", [D, BC * t_steps], FPR, False)
    Wc = nc.declare_dram_parameter("Wcat", [D, H3], FPR, False)
    Bc = nc.declare_dram_parameter("Bcat", [128, H3], FP, False)
    Rc = nc.declare_dram_parameter("Rcat", [H, H3], FPR, False)
    out = nc.declare_dram_parameter("out", [BC, H], FP, True)

    MT = (BC * t_steps) // 128   # 128-row tiles over (b, t)
    NT = H3 // 512               # 6 n-tiles of 512

    with TileContext(nc) as tc:
        with tc.tile_pool(name="const_pool", bufs=1) as cp:
            # identity [128,128] for full-tile transposes
            i128_t = cp.tile([128, 128], FP)
            make_identity(nc, i128_t[:])
            i128 = cp.tile([128, 128], FPR)
            nc.scalar.copy(out=i128[:], in_=i128_t[:])

            # ---------------- fused phase A + scan ----------------
            with (
                tc.tile_pool(name="scan_r", bufs=1) as rp,
                tc.tile_pool(name="xp_pool", bufs=2) as xpp,
                tc.tile_pool(name="sb_pool", bufs=2) as sbp,
                tc.tile_pool(name="ps_pool", bufs=1, space="PSUM") as psp,
            ):
                scan_body(nc, tc, rp, xpp, sbp, psp, i128,
                          xT, Wc, Bc, Rc, out, t_steps)
    nc.finalize()
    return nc


def scan_body(nc, tc, rp, xpp, sbp, psp, i128, xT, Wc, Bc, Rc, out, t_steps):
    R_sb = rp.tile([128, KH * H3], FPR)   # 96KB/partition, resident
    nc.sync.dma_start(
        out=R_sb[:],
        in_=Rc[:].rearrange("(kh p) n -> p kh n", kh=KH),
    )

    # phase-A residents: weights, bias, per-tile x/out rings
    bias_bc = rp.tile([128, H3], FP)
    nc.sync.dma_start(out=bias_bc[:], in_=Bc[:, :])
    w_sb = rp.tile([128, KD * H3], FPR)
    nc.sync.dma_start(
        out=w_sb[:], in_=Wc[:].rearrange("(kd p) n -> p kd n", kd=KD))
    xT_v = xT[:].rearrange("(kd p) m -> p kd m", kd=KD)
    MT = (BC * t_steps) // 128
    NT = H3 // 512

    # psum: gate pre-activations [16, 1024] batch-major (2 banks each),
    # one shared bank for the three H-major transpose targets, one spare
    pr = psp.tile([16, H], FP, tag="pr")
    pz = psp.tile([16, H], FP, tag="pz")
    ph = psp.tile([16, H], FP, tag="ph")
    tps = psp.tile([128, 512], FPR, tag="tps")
    tps_r, tps_z, tps_h = tps[:, 0:128], tps[:, 128:256], tps[:, 256:384]
    aps = psp.tile([128, 512], FP, tag="aps")
    gate_ps = {0: pr, 1: pz, 2: ph}
    for t_ in (pr, pz, ph):
        nc.vector.memset(t_[:], 0.0)

    x_tiles = {}
    xp_tiles = {}

    def x_dma(mt):
        x_sb = xpp.tile([128, KD * 128], FPR, tag="x_sb")
        nc.sync.dma_start(out=x_sb[:], in_=xT_v[:, :, mt * 128:(mt + 1) * 128])
        x_tiles[mt] = x_sb

    def pa_unit(u):
        """One phase-A unit: xp tile mt covers steps [8mt, 8mt+8) x 16 b;
        ntile = one 512-col slice of the 3H gate columns. The x@W+B result
        lands straight in SBUF - no DRAM round trip."""
        mt, ntile = u // NT, u % NT
        if ntile == 0:
            xp8 = xpp.tile([128, H3], FPR, tag="xp8")
            xp_tiles[mt] = xp8
            if mt + 1 < MT:
                x_dma(mt + 1)
        xp8 = xp_tiles[mt]
        x_sb = x_tiles[mt]
        for kd in range(KD):
            nc.tensor.matmul(
                aps[:], x_sb[:, kd * 128:(kd + 1) * 128],
                w_sb[:, kd * H3 + ntile * 512:kd * H3 + (ntile + 1) * 512],
                start=(kd == 0), stop=(kd == KD - 1),
            )
        nc.vector.tensor_tensor(
            xp8[:, ntile * 512:(ntile + 1) * 512], aps[:],
            bias_bc[:, ntile * 512:(ntile + 1) * 512], OP.add)

    zini = sbp.tile([128, 128], FP, tag="zini", bufs=1)
    nc.gpsimd.memset(zini[:], 0.0)
    sT0 = sbp.tile([128, 128], FPR, tag="sT")
    nc.vector.tensor_copy(sT0[:], zini[:])

    xp_tiles = {}

    def inject(t, gate):
        """Seed psum[gate] for step t with XP(+bias): 2 n-halves."""
        xp8 = xp_tiles[t // 8]
        ts = t % 8
        ps = gate_ps[gate]
        for nt in range(2):
            nc.tensor.matmul(
                ps[:, nt * 512:(nt + 1) * 512],
                i128[:, ts * 16:ts * 16 + 16],
                xp8[:, gate * H + nt * 512: gate * H + (nt + 1) * 512],
                start=True, stop=False,
            )

    def gate_mms(gate, stat, nt):
        """One 512-col half of gate += stat.T @ R[:, gate]; 8 K-chunks."""
        ps = gate_ps[gate]
        for k in range(KH):
            nc.tensor.matmul(
                ps[:, nt * 512:(nt + 1) * 512],
                stat[:, 16 * k:16 * k + 16],
                R_sb[:, k * H3 + gate * H + nt * 512:
                     k * H3 + gate * H + (nt + 1) * 512],
                start=False, stop=(k == KH - 1),
            )

    def transp(dst, src_sb, half):
        """4 chunk transposes of one 512-col half into H-major dst."""
        for k in range(4 * half, 4 * half + 4):
            nc.tensor.transpose(
                dst[:, 16 * k:16 * k + 16],
                src_sb[0:16, 128 * k:128 * (k + 1)],
                i128[0:16, 0:16],
            )

    # ---- prologue: xp tiles for the first two 8-step windows ----
    total_units = MT * NT
    prologue_units = min(NT, total_units)
    x_dma(0)
    for u in range(prologue_units):
        pa_unit(u)
    au = prologue_units

    sT = sT0
    for gate in range(3):
        inject(0, gate)

    for t in range(t_steps):
        last = t + 1 >= t_steps
        # ---- r gate ----
        gate_mms(0, sT, 0)
        r_sb = sbp.tile([16, H], FPR, tag="g_sb", bufs=1)
        nc.scalar.activation(r_sb[:, 0:512], pr[:, 0:512], AF.Sigmoid)
        gate_mms(0, sT, 1)
        nc.scalar.activation(r_sb[:, 512:], pr[:, 512:], AF.Sigmoid)
        transp(tps_r, r_sb, 0)
        transp(tps_r, r_sb, 1)
        rs = sbp.tile([128, 128], FPR, tag="rs")
        for hf in range(2):
            sl = slice(64 * hf, 64 * (hf + 1))
            nc.vector.tensor_tensor(rs[:, sl], _f(tps_r)[:, sl], _f(sT)[:, sl],
                                    OP.mult)

        # ---- z gate ----
        gate_mms(1, sT, 0)
        z_sb = sbp.tile([16, H], FPR, tag="g_sb", bufs=1)
        nc.scalar.activation(z_sb[:, 0:512], pz[:, 0:512], AF.Sigmoid)
        gate_mms(1, sT, 1)
        nc.scalar.activation(z_sb[:, 512:], pz[:, 512:], AF.Sigmoid)
        transp(tps_z, z_sb, 0)
        transp(tps_z, z_sb, 1)
        zT = sbp.tile([128, 128], FP, tag="zT")
        nc.vector.tensor_copy(zT[:], _f(tps_z))
        zs = sbp.tile([128, 128], FP, tag="zs")
        nc.vector.tensor_tensor(zs[:], zT[:], _f(sT)[:], OP.mult)
        u = sbp.tile([128, 128], FP, tag="u")
        nc.vector.tensor_tensor(u[:], _f(sT)[:], zs[:], OP.subtract)

        # ---- h gate (needs rs) ----
        gate_mms(2, rs, 0)
        h_sb = sbp.tile([16, H], FPR, tag="g_sb", bufs=1)
        nc.scalar.activation(h_sb[:, 0:512], ph[:, 0:512], AF.Tanh)
        gate_mms(2, rs, 1)
        nc.scalar.activation(h_sb[:, 512:], ph[:, 512:], AF.Tanh)

        # keep the PE streaming through the state-update tail: next step's
        # r/z injects, then interleaved phase-A units, land in the PE queue
        # before the transposes that wait on tanh/DVE
        if not last:
            inject(t + 1, 0)
            inject(t + 1, 1)
        target = min(total_units, (t // 8 + 1) * NT + min(t % 8 + 1, NT))
        while au < target:
            pa_unit(au)
            au += 1

        sT_new = sbp.tile([128, 128], FPR, tag="sT")
        zh = sbp.tile([128, 128], FP, tag="zh")
        for hf in range(2):
            sl64 = slice(64 * hf, 64 * (hf + 1))
            transp(tps_h, h_sb, hf)
            nc.vector.tensor_tensor(zh[:, sl64], _f(tps_h)[:, sl64],
                                    zT[:, sl64], OP.mult)
            nc.vector.tensor_tensor(sT_new[:, sl64], u[:, sl64], zh[:, sl64],
                                    OP.add)
        if not last:
            inject(t + 1, 2)
        sT = sT_new

    # ---- epilogue: H-major state -> batch-major output ----
    ops0 = psp.tile([16, 512], FPR, tag="aps")
    ops1 = psp.tile([16, 512], FPR, tag="aps")
    out_sb = sbp.tile([16, H], FP, tag="g_sb", bufs=1)
    for k in range(KH):
        dst = (ops0 if k < 4 else ops1)
        nc.tensor.transpose(
            dst[0:16, 128 * (k % 4):128 * (k % 4 + 1)],
            sT[:, 16 * k:16 * k + 16], i128[:],
        )
        if k == 3:
            nc.vector.tensor_copy(out_sb[:, 0:512], _f(ops0)[0:16, :])
    nc.vector.tensor_copy(out_sb[:, 512:1024], _f(ops1)[0:16, :])
    nc.sync.dma_start(out=out[:, :], in_=out_sb[:])


_CACHE = {}


def _get_nc(t_steps=T):
    key = t_steps
    if key not in _CACHE:
        _CACHE[key] = build_gru(t_steps)
    return _CACHE[key]


def prep_in_maps(inputs, t_steps=T):
    x = np.asarray(inputs["x"], dtype=np.float32)
    Wcat = np.ascontiguousarray(
        np.concatenate([np.asarray(inputs["W_r"]), np.asarray(inputs["W_z"]),
                        np.asarray(inputs["W_h"])], axis=1),
        dtype=np.float32,
    )
    Rcat = np.ascontiguousarray(
        np.concatenate([np.asarray(inputs["R_r"]), np.asarray(inputs["R_z"]),
                        np.asarray(inputs["R_h"])], axis=1),
        dtype=np.float32,
    )
    Bcat = np.ascontiguousarray(
        np.broadcast_to(
            np.concatenate([np.asarray(inputs["B_r"]), np.asarray(inputs["B_z"]),
                            np.asarray(inputs["B_h"])])[None, :], (128, H3)),
        dtype=np.float32,
    )
    in_maps = []
    for c in range(NCORES):
        xc = x[c * BC:(c + 1) * BC, :t_steps, :]          # [BC, t, D]
        xTc = np.ascontiguousarray(
            xc.transpose(2, 1, 0).reshape(D, BC * t_steps)
        )
        in_maps.append({"xT": xTc, "Wcat": Wcat, "Bcat": Bcat, "Rcat": Rcat})
    return in_maps


def assemble_output(results):
    outs = [results[c]["out"] for c in range(NCORES)]
    return np.concatenate(outs, axis=0)


def kernel_run(x, W_z, W_r, W_h, R_z, R_r, R_h, B_z, B_r, B_h, t_steps=T, **run_kw):
    in_maps = prep_in_maps(dict(x=x, W_z=W_z, W_r=W_r, W_h=W_h, R_z=R_z, R_r=R_r,
                                R_h=R_h, B_z=B_z, B_r=B_r, B_h=B_h), t_steps)
    res = run_bass_kernel_spmd(_get_nc(t_steps), in_maps, list(range(NCORES)), **run_kw)
    full = assemble_output(res.results)
    return full, res


def kernel(**inputs):
    full, _ = kernel_run(**inputs)
    return full
